# revision 25
# baseline (speedup 1.0000x reference)
"""Trainium2 Bass kernel for nn_EICLayer2 (gnn_message_passing).

Computation (per batch element b):
  rows 0-2: for each (row, col2): y[b,row,col2,:] = sigmoid(z - 0.5*max_g(z))
            where z = chunk[b,row,col2,:] @ W256[row*4+col2].T
            and chunk[...,l1c*64+k] = x[b,row,l1c,col2*64+k]
  row 3:    same with only l1c in {0,1,2} (192 input features), W192.

Strategy: pure data-parallel over batch across 8 cores (2048 each).
Per core, per 128-batch tile (4 groups of 4 (row,col2) chunks):
  DMA x (fp32, first 3840 of 4096 features) -> cast+swizzle to fp16 on GpSimd
  -> phase 1: all 32 PE transposes + 4 batched DVE copybacks PSUM->SBUF
  -> phase 2 per group: 8 accumulating fp16 matmuls against host-prescaled
  W' = -0.5*W^T (so z' = -0.5 z and z - 0.5 max z == -2 z' + min z')
  -> DVE reduce_min -> ACT sigmoid with scale=-2, bias=min -> DMA out
  (fp16, host-upcast).

The phase split keeps DVE packed (copybacks of tile t+1 don't sit behind
reduces of tile t in DVE program order).

Weights are tiny (<4MB); pre-transposed/scaled/padded to fp16 on host and
replicated to all cores.
"""

import numpy as np

B = 16384
N_CORES = 8
B_CORE = B // N_CORES  # 2048
P = 128

# knobs for experimentation
TRACE = False
STITCH = False
LAST_RESULTS = None  # BassKernelResults of last run


def _build_bass(b_core=B_CORE, variant="full"):
    import concourse.mybir as mybir
    import concourse.tile as tile
    from concourse import bacc
    from concourse.bass import ts
    from concourse.masks import make_identity

    fp32 = mybir.dt.float32
    fp16 = mybir.dt.float16

    n_tiles = b_core // P
    n_prefetch = 2  # x tiles DMA'd ahead of the weight load

    nc = bacc.Bacc("TRN2", target_bir_lowering=False, debug=False)
    x_d = nc.dram_tensor("x", [b_core, 4, 4, 256], fp32, kind="ExternalInput")
    # host pre-swizzled: wt_d[p, rc, j, g] = -0.5 * W^T[rc][j*128+p, g]
    wt_d = nc.dram_tensor("wt", [P, 16, 2, 256], fp16, kind="ExternalInput")
    y_d = nc.dram_tensor("y", [b_core, 4, 4, 256], fp16, kind="ExternalOutput")

    x_tiled = x_d.rearrange("(t p) r c f -> t p (r c f)", p=P)  # [T, 128, 4096]
    y_tiled = y_d.rearrange("(t p) r c f -> t p (r c f)", p=P)

    with tile.TileContext(nc) as tc:
        with (
            tc.tile_pool(name="singles", bufs=1) as singles,
            tc.tile_pool(name="xin", bufs=4) as xin_pool,
            tc.tile_pool(name="x16", bufs=2) as x16_pool,
            tc.tile_pool(name="xt", bufs=10) as xt_pool,
            tc.tile_pool(name="yout", bufs=3) as y_pool,
            tc.tile_pool(name="mn", bufs=8) as mn_pool,
            tc.tile_pool(name="py", bufs=4, space="PSUM") as py_pool,
        ):
            # prefetch first x tiles interleaved with quarter-loads of the
            # weights (group g only needs weight rows 4g..4g+3) so tile-0
            # group-0 matmuls can start as early as possible
            ident = singles.tile([P, P], fp16)
            make_identity(nc, ident)
            wt_sb = singles.tile([P, 16, 2, 256], fp16)

            x32_pre = []
            x32 = xin_pool.tile([P, 3840], fp32, name="x32")
            nc.sync.dma_start(out=x32, in_=x_tiled[0][:, 0:3840])
            x32_pre.append(x32)
            nc.sync.dma_start(out=wt_sb[:, 0:4], in_=wt_d[:, 0:4])
            x32 = xin_pool.tile([P, 3840], fp32, name="x32")
            nc.sync.dma_start(out=x32, in_=x_tiled[1][:, 0:3840])
            x32_pre.append(x32)
            for g in range(1, 4):
                nc.sync.dma_start(
                    out=wt_sb[:, 4 * g : 4 * g + 4], in_=wt_d[:, 4 * g : 4 * g + 4]
                )

            def phase1_group(x16, grp):
                # Transposes write into the fp16-bitcast FIRST HALF of the
                # same py tile the group's matmuls will later overwrite: the
                # copyback->matmul dependency already serializes that reuse,
                # so no separate pt PSUM pool is needed and py gets 4 bufs.
                py = py_pool.tile([P, 4, 256], fp32)
                ptv = py[:, 0:2, :].bitcast(fp16).rearrange("p a b -> p (a b)")
                for i in range(4):
                    rc = grp * 4 + i
                    r, c = rc // 4, rc % 4
                    for j in range(2):
                        nc.tensor.transpose(
                            ptv[:, ts(2 * i + j, P)],
                            x16[:, r, c, ts(j, P)],
                            ident,
                        )
                xt = xt_pool.tile([P, 4, 2, P], fp16)
                nc.vector.tensor_copy(
                    out=xt.rearrange("p i j b -> p (i j b)"), in_=ptv
                )
                return xt, py

            def phase2_group(xt_py, y_sb, grp):
                # matmuls + reduce + sigmoid for one group of 4 chunks
                xt, py = xt_py
                for i in range(4):
                    rc = grp * 4 + i
                    nc.tensor.matmul(
                        py[:, i, :], xt[:, i, 0, :], wt_sb[:, rc, 0, :],
                        start=True, stop=False, skip_group_check=True,
                    )
                    nc.tensor.matmul(
                        py[:, i, :], xt[:, i, 1, :], wt_sb[:, rc, 1, :],
                        start=False, stop=True, skip_group_check=True,
                    )
                # py = -0.5*z, so z - 0.5*max(z) == -2*py + min(py)
                mn = mn_pool.tile([P, 4], fp32, tag="mn")
                nc.vector.tensor_reduce(
                    out=mn, in_=py, axis=mybir.AxisListType.X,
                    op=mybir.AluOpType.min,
                )
                for i in range(4):
                    rc = grp * 4 + i
                    nc.scalar.activation(
                        out=y_sb[:, ts(rc, 256)],
                        in_=py[:, i, :],
                        func=mybir.ActivationFunctionType.Sigmoid,
                        bias=mn[:, i : i + 1],
                        scale=-2.0,
                    )

            # software-pipelined by one tile at GROUP granularity: the PE
            # stream alternates transposes (tile t) with matmuls (tile t-1)
            # so DVE always has a copyback and a reduce ready back-to-back
            xts_prev = None
            for t in range(n_tiles):
                if t < n_prefetch:
                    x32 = x32_pre[t]
                else:
                    # skip the unused (row3, l1c3) chunk: contiguous 3840 prefix
                    x32 = xin_pool.tile([P, 3840], fp32)
                    nc.sync.dma_start(out=x32, in_=x_tiled[t][:, 0:3840])

                # x16[p, r, c, l*64+k] = x32[p, r*1024 + l*256 + c*64 + k]
                x16 = x16_pool.tile([P, 4, 4, 256], fp16)
                x32v = x32[:, 0:3072].rearrange(
                    "p (r l c k) -> p r c l k", r=3, l=4, c=4
                )
                x32v3 = x32[:, 3072:3840].rearrange(
                    "p (l c k) -> p c l k", l=3, c=4
                )
                for r in range(4):
                    nl = 4 if r < 3 else 3
                    src = x32v[:, r] if r < 3 else x32v3
                    dst = x16[:, r, :, 0 : nl * 64].rearrange(
                        "p c (l k) -> p c l k", l=nl
                    )
                    nc.gpsimd.tensor_copy(out=dst, in_=src)
                if t < 2:
                    # zero the (row3, l1c3) feature lanes once per buffer so
                    # transposed garbage can't poison the zero-weight matmul
                    # rows; nothing ever overwrites this region afterwards
                    nc.gpsimd.memset(x16[:, 3, :, 192:256], 0.0)

                xts = []
                y_sb = (
                    y_pool.tile([P, 4096], fp16, name="y_sb")
                    if xts_prev
                    else None
                )
                for grp in range(4):
                    xts.append(phase1_group(x16, grp))
                    if xts_prev is not None:
                        # boost matmul/reduce/sigmoid priority so the
                        # scheduler feeds DVE's reduces (the bottleneck
                        # engine) instead of bunching transposes first
                        phase2_group(xts_prev[grp], y_sb, grp)
                if xts_prev is not None:
                    nc.scalar.dma_start(out=y_tiled[t - 1], in_=y_sb)
                xts_prev = xts

            # epilogue: last tile's compute
            y_sb = y_pool.tile([P, 4096], fp16)
            for grp in range(4):
                phase2_group(xts_prev[grp], y_sb, grp)
            nc.scalar.dma_start(out=y_tiled[n_tiles - 1], in_=y_sb)
    nc.compile()
    return nc


def _prep_weights(W256, W192):
    wt = np.zeros((16, 256, 256), np.float16)
    w256 = np.asarray(W256, np.float32).reshape(3, 4, 256, 256)  # [r, c, g, f]
    for r in range(3):
        for c in range(4):
            wt[r * 4 + c] = (-0.5 * w256[r, c].T).astype(np.float16)  # [f, g]
    w192 = np.asarray(W192, np.float32)  # [c, g, f]
    for c in range(4):
        wt[12 + c, 0:192, :] = (-0.5 * w192[c].T).astype(np.float16)
    # swizzle to DMA-friendly layout: [p, rc, j, g] = wt[rc, j*128+p, g]
    return np.ascontiguousarray(wt.reshape(16, 2, P, 256).transpose(2, 0, 1, 3))


def _in_maps(x, W256, W192):
    x = np.ascontiguousarray(np.asarray(x, np.float32))
    wt = _prep_weights(W256, W192)
    return [
        {"x": x[i * B_CORE : (i + 1) * B_CORE], "wt": wt}
        for i in range(N_CORES)
    ]


def kernel(x, W256, W192):
    global LAST_RESULTS
    from concourse.bass_utils import run_bass_kernel_spmd

    nc = _build_bass()
    res = run_bass_kernel_spmd(
        nc,
        _in_maps(x, W256, W192),
        core_ids=list(range(N_CORES)),
        trace=TRACE,
        stitch_traces=STITCH,
    )
    LAST_RESULTS = res
    out = np.concatenate([r["y"] for r in res.results], axis=0)
    # y is stored fp16 on-chip to halve output DMA traffic; upcast on host
    return out.astype(np.float32)


# revision 32
# speedup vs baseline: 5.1626x; 5.1626x over previous
"""Trainium2 Bass kernel for nn_EICLayer2 (gnn_message_passing).

Computation (per batch element b):
  rows 0-2: for each (row, col2): y[b,row,col2,:] = sigmoid(z - 0.5*max_g(z))
            where z = chunk[b,row,col2,:] @ W256[row*4+col2].T
            and chunk[...,l1c*64+k] = x[b,row,l1c,col2*64+k]
  row 3:    same with only l1c in {0,1,2} (192 input features), W192.

Strategy: pure data-parallel over batch across 8 cores (2048 each).
Per core, per 128-batch tile (4 groups of 4 (row,col2) chunks):
  DMA x (fp32, first 3840 of 4096 features) -> cast+swizzle to fp16 on GpSimd
  -> phase 1: all 32 PE transposes + 4 batched DVE copybacks PSUM->SBUF
  -> phase 2 per group: 8 accumulating fp16 matmuls against host-prescaled
  W' = -0.5*W^T (so z' = -0.5 z and z - 0.5 max z == -2 z' + min z')
  -> DVE reduce_min -> ACT sigmoid with scale=-2, bias=min -> DMA out
  (fp16, host-upcast).

The phase split keeps DVE packed (copybacks of tile t+1 don't sit behind
reduces of tile t in DVE program order).

Weights are tiny (<4MB); pre-transposed/scaled/padded to fp16 on host and
replicated to all cores.
"""

import numpy as np

B = 16384
N_CORES = 8
B_CORE = B // N_CORES  # 2048
P = 128

# knobs for experimentation
TRACE = False
STITCH = False
LAST_RESULTS = None  # BassKernelResults of last run


def _build_bass(b_core=B_CORE, variant="full", n_reps=1):
    import concourse.mybir as mybir
    import concourse.tile as tile
    from concourse import bacc
    from concourse.bass import ts
    from concourse.masks import make_identity

    fp32 = mybir.dt.float32
    fp16 = mybir.dt.float16

    n_tiles = b_core // P
    n_prefetch = 2  # x tiles DMA'd ahead of the weight load

    nc = bacc.Bacc("TRN2", target_bir_lowering=False, debug=False)
    x_d = nc.dram_tensor("x", [b_core, 4, 4, 256], fp32, kind="ExternalInput")
    # host pre-swizzled: wt_d[p, rc, j, g] = -0.5 * W^T[rc][j*128+p, g]
    wt_d = nc.dram_tensor("wt", [P, 16, 2, 256], fp16, kind="ExternalInput")
    y_d = nc.dram_tensor("y", [b_core, 4, 4, 256], fp16, kind="ExternalOutput")

    x_tiled = x_d.rearrange("(t p) r c f -> t p (r c f)", p=P)  # [T, 128, 4096]
    y_tiled = y_d.rearrange("(t p) r c f -> t p (r c f)", p=P)

    with tile.TileContext(nc) as tc:
        with (
            tc.tile_pool(name="singles", bufs=1) as singles,
            tc.tile_pool(name="xin", bufs=4) as xin_pool,
            tc.tile_pool(name="x16", bufs=2) as x16_pool,
            tc.tile_pool(name="xt", bufs=10) as xt_pool,
            tc.tile_pool(name="yout", bufs=3) as y_pool,
            tc.tile_pool(name="mn", bufs=8) as mn_pool,
            tc.tile_pool(name="py", bufs=4, space="PSUM") as py_pool,
        ):
            # prefetch first x tiles interleaved with quarter-loads of the
            # weights (group g only needs weight rows 4g..4g+3) so tile-0
            # group-0 matmuls can start as early as possible
            ident = singles.tile([P, P], fp16)
            make_identity(nc, ident)
            # touch Sigmoid immediately so the ~2.7us ACT table load runs
            # during the initial x DMA instead of before tile-0's first
            # sigmoid on the critical path
            warm_sig = singles.tile([P, 4], fp16)
            nc.scalar.activation(
                out=warm_sig,
                in_=ident[:, 0:4],
                func=mybir.ActivationFunctionType.Sigmoid,
            )
            wt_sb = singles.tile([P, 16, 2, 256], fp16)

            x32_pre = []
            if n_reps == 1:
                # first tile's x arrives as two half-DMAs so its swizzle can
                # start ~2.7us earlier
                x32 = xin_pool.tile([P, 3840], fp32, name="x32")
                nc.sync.dma_start(out=x32[:, 0:2048], in_=x_tiled[0][:, 0:2048])
                nc.sync.dma_start(
                    out=x32[:, 2048:3840], in_=x_tiled[0][:, 2048:3840]
                )
                x32_pre.append(x32)
                nc.sync.dma_start(out=wt_sb[:, 0:4], in_=wt_d[:, 0:4])
                x32 = xin_pool.tile([P, 3840], fp32, name="x32")
                nc.sync.dma_start(out=x32, in_=x_tiled[1][:, 0:3840])
                x32_pre.append(x32)
                for g in range(1, 4):
                    nc.sync.dma_start(
                        out=wt_sb[:, 4 * g : 4 * g + 4],
                        in_=wt_d[:, 4 * g : 4 * g + 4],
                    )
            else:
                nc.sync.dma_start(out=wt_sb, in_=wt_d[:])

            def phase1_group(x16, grp):
                # Transposes write into the fp16-bitcast FIRST HALF of the
                # same py tile the group's matmuls will later overwrite: the
                # copyback->matmul dependency already serializes that reuse,
                # so no separate pt PSUM pool is needed and py gets 4 bufs.
                py = py_pool.tile([P, 4, 256], fp32)
                ptv = py[:, 0:2, :].bitcast(fp16).rearrange("p a b -> p (a b)")
                for i in range(4):
                    rc = grp * 4 + i
                    r, c = rc // 4, rc % 4
                    for j in range(2):
                        nc.tensor.transpose(
                            ptv[:, ts(2 * i + j, P)],
                            x16[:, r, c, ts(j, P)],
                            ident,
                        )
                xt = xt_pool.tile([P, 4, 2, P], fp16)
                nc.vector.tensor_copy(
                    out=xt.rearrange("p i j b -> p (i j b)"), in_=ptv
                )
                return xt, py

            def phase2_group(xt_py, y_sb, grp):
                # matmuls + reduce + sigmoid for one group of 4 chunks
                xt, py = xt_py
                for i in range(4):
                    rc = grp * 4 + i
                    nc.tensor.matmul(
                        py[:, i, :], xt[:, i, 0, :], wt_sb[:, rc, 0, :],
                        start=True, stop=False, skip_group_check=True,
                    )
                    nc.tensor.matmul(
                        py[:, i, :], xt[:, i, 1, :], wt_sb[:, rc, 1, :],
                        start=False, stop=True, skip_group_check=True,
                    )
                # py = -0.5*z, so z - 0.5*max(z) == -2*py + min(py)
                mn = mn_pool.tile([P, 4], fp32, tag="mn")
                nc.vector.tensor_reduce(
                    out=mn, in_=py, axis=mybir.AxisListType.X,
                    op=mybir.AluOpType.min,
                )
                for i in range(4):
                    rc = grp * 4 + i
                    nc.scalar.activation(
                        out=y_sb[:, ts(rc, 256)],
                        in_=py[:, i, :],
                        func=mybir.ActivationFunctionType.Sigmoid,
                        bias=mn[:, i : i + 1],
                        scale=-2.0,
                    )

            def emit_tiles(x32_pre):
                # software-pipelined by one tile at GROUP granularity: the PE
                # stream alternates transposes (tile t) with matmuls (tile
                # t-1) so DVE always has a copyback and a reduce ready
                xts_prev = None
                for t in range(n_tiles):
                    if t < len(x32_pre):
                        x32 = x32_pre[t]
                    else:
                        # skip the unused (row3,l1c3) chunk: 3840-elem prefix
                        x32 = xin_pool.tile([P, 3840], fp32, name="x32")
                        nc.sync.dma_start(out=x32, in_=x_tiled[t][:, 0:3840])

                    # x16[p,r,c,l*64+k] = x32[p, r*1024 + l*256 + c*64 + k]
                    x16 = x16_pool.tile([P, 4, 4, 256], fp16, name="x16")
                    x32v = x32[:, 0:3072].rearrange(
                        "p (r l c k) -> p r c l k", r=3, l=4, c=4
                    )
                    x32v3 = x32[:, 3072:3840].rearrange(
                        "p (l c k) -> p c l k", l=3, c=4
                    )
                    for r in range(4):
                        nl = 4 if r < 3 else 3
                        src = x32v[:, r] if r < 3 else x32v3
                        dst = x16[:, r, :, 0 : nl * 64].rearrange(
                            "p c (l k) -> p c l k", l=nl
                        )
                        nc.gpsimd.tensor_copy(out=dst, in_=src)
                    if t < 2:
                        # zero the (row3, l1c3) feature lanes once per buffer
                        # so transposed garbage can't poison the zero-weight
                        # matmul rows; nothing overwrites this region after
                        nc.gpsimd.memset(x16[:, 3, :, 192:256], 0.0)

                    xts = []
                    y_sb = (
                        y_pool.tile([P, 4096], fp16, name="y_sb")
                        if xts_prev
                        else None
                    )
                    for grp in range(4):
                        xts.append(phase1_group(x16, grp))
                        if xts_prev is not None:
                            phase2_group(xts_prev[grp], y_sb, grp)
                    if xts_prev is not None:
                        nc.scalar.dma_start(out=y_tiled[t - 1], in_=y_sb)
                    xts_prev = xts

                # epilogue: last tile's compute
                y_sb = y_pool.tile([P, 4096], fp16, name="y_sb")
                for grp in range(4):
                    phase2_group(xts_prev[grp], y_sb, grp)
                nc.scalar.dma_start(out=y_tiled[n_tiles - 1], in_=y_sb)

            if n_reps == 1:
                emit_tiles(x32_pre)
            else:
                # timing mode: repeat the whole pipeline on-device so wall
                # time can be sloped over n_reps through the noisy tunnel
                with tc.For_i(0, n_reps):
                    emit_tiles([])
    nc.compile()
    return nc


def _prep_weights(W256, W192):
    wt = np.zeros((16, 256, 256), np.float16)
    w256 = np.asarray(W256, np.float32).reshape(3, 4, 256, 256)  # [r, c, g, f]
    for r in range(3):
        for c in range(4):
            wt[r * 4 + c] = (-0.5 * w256[r, c].T).astype(np.float16)  # [f, g]
    w192 = np.asarray(W192, np.float32)  # [c, g, f]
    for c in range(4):
        wt[12 + c, 0:192, :] = (-0.5 * w192[c].T).astype(np.float16)
    # swizzle to DMA-friendly layout: [p, rc, j, g] = wt[rc, j*128+p, g]
    return np.ascontiguousarray(wt.reshape(16, 2, P, 256).transpose(2, 0, 1, 3))


def _in_maps(x, W256, W192):
    x = np.ascontiguousarray(np.asarray(x, np.float32))
    wt = _prep_weights(W256, W192)
    return [
        {"x": x[i * B_CORE : (i + 1) * B_CORE], "wt": wt}
        for i in range(N_CORES)
    ]


def kernel(x, W256, W192):
    global LAST_RESULTS
    from concourse.bass_utils import run_bass_kernel_spmd

    nc = _build_bass()
    res = run_bass_kernel_spmd(
        nc,
        _in_maps(x, W256, W192),
        core_ids=list(range(N_CORES)),
        trace=TRACE,
        stitch_traces=STITCH,
    )
    LAST_RESULTS = res
    out = np.concatenate([r["y"] for r in res.results], axis=0)
    # y is stored fp16 on-chip to halve output DMA traffic; upcast on host
    return out.astype(np.float32)
